# revision 1
# baseline (speedup 1.0000x reference)
"""MAE ViT-Base encoder (masked, KEEP=49) on 8 TRN2 NeuronCores.

Data-parallel over batch (8 images/core). Feature-major activations
[768, 392] on-chip; bf16 matmul inputs, fp32 PSUM accumulation, fp32
residual stream. Only the 49 kept patches are embedded (the mask gather
happens on host before the conv matmul). LayerNorm statistics and
per-token broadcasts are computed with ones/rank-1 matmuls on the
TensorEngine; softmax is computed in transposed layout so attention
needs no on-chip transposes.
"""

import numpy as np
import ml_dtypes

import concourse.bass as bass
import concourse.mybir as mybir
import concourse.tile as tile
from concourse import bacc
from concourse.bass import ts
from concourse.bass_utils import run_bass_kernel_spmd

AFT = mybir.ActivationFunctionType
BF16 = mybir.dt.bfloat16
F32 = mybir.dt.float32
ALU = mybir.AluOpType

B, P, DIM, DEPTH, NH, DH, FF = 64, 16, 768, 12, 12, 64, 3072
KEEP = 49
NCORES = 8
BL = B // NCORES          # 8 images per core
T = BL * KEEP             # 392 tokens per core
C = DIM // 128            # 6 feature chunks
FC = FF // 128            # 24 ffn chunks
EPS = 1e-9

bf16 = ml_dtypes.bfloat16
_cached = {}


def _chunk_pack(w, cols):
    """[768, cols] -> [128, 6*cols] with tile[p, c*cols+x] = w[c*128+p, x]."""
    return np.ascontiguousarray(
        w.reshape(C, 128, cols).transpose(1, 0, 2).reshape(128, C * cols))


def _build():
    nc = bacc.Bacc("TRN2", target_bir_lowering=False, debug=False,
                   enable_asserts=False, num_devices=NCORES)

    def din(name, shape, dt=BF16):
        return nc.dram_tensor(name, shape, dt, kind="ExternalInput").ap()

    pxT = din("pxT", [128, C * T])
    convw = din("convw", [128, C * DIM])
    peL = din("peL", [KEEP, DIM])
    repI = din("repI", [KEEP, T])
    qw_d = din("qw", [DEPTH, 128, C * DIM])
    kw_d = din("kw", [DEPTH, 128, C * DIM])
    vw_d = din("vw", [DEPTH, 128, C * DIM])
    pw_d = din("pw", [DEPTH, 128, C * DIM])
    w1_d = din("w1q", [DEPTH, 4, 128, C * DIM])
    w2_d = din("w2q", [DEPTH, 128, 4 * C * DIM])
    b2_d = din("b2p", [DEPTH, 1, DIM])
    b1_d = din("b1c", [DEPTH, 128, FC], F32)
    ln1r_d = din("ln1r", [DEPTH, 1, DIM])   # ln1 scale rows
    ln1b_d = din("ln1b", [DEPTH, 1, DIM])   # -ln1 bias rows
    ln2r_d = din("ln2r", [DEPTH, 1, DIM])
    ln2b_d = din("ln2b", [DEPTH, 1, DIM])
    out_d = nc.dram_tensor("out", [DIM, T], F32, kind="ExternalOutput").ap()

    with tile.TileContext(nc) as tc:
        from contextlib import ExitStack
        es = ExitStack()
        cpool = es.enter_context(tc.tile_pool(name="consts", bufs=1))
        apool = es.enter_context(tc.tile_pool(name="acts", bufs=1))
        hpool = es.enter_context(tc.tile_pool(name="h", bufs=1))
        vpool = es.enter_context(tc.tile_pool(name="vt", bufs=5))
        wq_pool = es.enter_context(tc.tile_pool(name="wqkv", bufs=2))
        wp_pool = es.enter_context(tc.tile_pool(name="wp", bufs=1))
        w1_pool = es.enter_context(tc.tile_pool(name="w1", bufs=2))
        w2_pool = es.enter_context(tc.tile_pool(name="w2", bufs=1))
        lp_pool = es.enter_context(tc.tile_pool(name="lparam", bufs=2))
        tpool = es.enter_context(tc.tile_pool(name="tmp", bufs=3))
        r1pool = es.enter_context(tc.tile_pool(name="rows1", bufs=1))
        r2pool = es.enter_context(tc.tile_pool(name="rows2", bufs=2))
        epool = es.enter_context(tc.tile_pool(name="etiles", bufs=8))
        pspool = es.enter_context(tc.tile_pool(name="ps", bufs=8, space="PSUM"))

        def psum(shape=(128, T)):
            return pspool.tile(list(shape), F32, tag="ps", name="ps")

        # ---- small constants
        ones128 = cpool.tile([128, 1], BF16, tag="ones128", name="ones128")
        nc.vector.memset(ones128[:], 1.0)
        ones49 = cpool.tile([KEEP, 1], BF16, tag="ones49", name="ones49")
        nc.vector.memset(ones49[:], 1.0)
        onesr1 = cpool.tile([1, 128], BF16, tag="onesr1", name="onesr1")
        nc.vector.memset(onesr1[:], 1.0)
        onesrT = cpool.tile([1, T], BF16, tag="onesrT", name="onesrT")
        nc.vector.memset(onesrT[:], 1.0)
        eps1 = cpool.tile([1, 1], F32, tag="eps1", name="eps1")
        nc.vector.memset(eps1[:], EPS)
        zer49 = cpool.tile([KEEP, 1], F32, tag="zer49", name="zer49")
        nc.vector.memset(zer49[:], 0.0)

        # ---- patch-embed constants share the w2 slot (freed before layer 0 FFN)
        NPX, NCW, NPE, NRI = C * T, C * DIM, DIM, T
        cst = w2_pool.tile([128, NPX + NCW + NPE + NRI], BF16, tag="w2", name="w2")
        px_sb = cst[:, 0:NPX]
        cw_sb = cst[:, NPX:NPX + NCW]
        pe_sb = cst[0:KEEP, NPX + NCW:NPX + NCW + NPE]
        ri_sb = cst[0:KEEP, NPX + NCW + NPE:NPX + NCW + NPE + NRI]
        nc.sync.dma_start(px_sb, pxT)
        nc.sync.dma_start(cw_sb, convw)
        nc.sync.dma_start(pe_sb, peL)
        nc.sync.dma_start(ri_sb, repI)

        x = apool.tile([128, C * T], F32, tag="x", name="x")
        qs = apool.tile([128, C * T], BF16, tag="qs", name="qs")
        ks = apool.tile([128, C * T], BF16, tag="ks", name="ks")
        aO = apool.tile([128, C * T], BF16, tag="aO", name="aO")
        g = apool.tile([128, FC * T], BF16, tag="g", name="g")

        def ln_stats_chunk(st, c):
            ps_sx, ps_sxx = st
            xbc = tpool.tile([128, T], BF16, tag="xb", name="xb")
            nc.vector.tensor_copy(xbc[:], x[:, ts(c, T)])
            x2c = tpool.tile([128, T], BF16, tag="x2", name="x2")
            nc.vector.tensor_mul(x2c[:], xbc[:], xbc[:])
            nc.tensor.matmul(ps_sx[:], ones128[:], xbc[:],
                             start=(c == 0), stop=(c == C - 1))
            nc.tensor.matmul(ps_sxx[:], ones128[:], x2c[:],
                             start=(c == 0), stop=(c == C - 1))

        def layernorm(ln_r, ln_b, h):
            st = (psum((1, T)), psum((1, T)))
            for c in range(C):
                ln_stats_chunk(st, c)
            ps_sx, ps_sxx = st
            m_sb = r2pool.tile([1, T], F32, tag="m_sb", name="m_sb")
            nc.scalar.mul(m_sb[:], ps_sx[:], 1.0 / DIM)
            msq = r1pool.tile([1, T], F32, tag="msq", name="msq")
            nc.vector.tensor_mul(msq[:], m_sb[:], m_sb[:])
            var = r1pool.tile([1, T], F32, tag="var", name="var")
            nc.vector.scalar_tensor_tensor(var[:], ps_sxx[:], 1.0 / DIM, msq[:],
                                           ALU.mult, ALU.subtract)
            sd = r1pool.tile([1, T], F32, tag="sd", name="sd")
            nc.scalar.activation(sd[:], var[:], AFT.Sqrt, bias=eps1[:])
            rstd_b = r2pool.tile([1, T], BF16, tag="rstd_b", name="rstd_b")
            with nc.allow_low_precision(reason="bf16 LN broadcast"):
                nc.vector.reciprocal(rstd_b[:], sd[:])
            mc_b = r2pool.tile([1, T], BF16, tag="mc_b", name="mc_b")
            nc.vector.tensor_mul(mc_b[:], m_sb[:], rstd_b[:])
            # h = x * (s x rstd) - (s x (m*rstd) + (-b) x 1)
            for c in range(C):
                ps_rs = psum()
                nc.tensor.matmul(ps_rs[:], ln_r[:, ts(c, 128)], rstd_b[:],
                                 start=True, stop=True)
                ps_mc = psum()
                nc.tensor.matmul(ps_mc[:], ln_r[:, ts(c, 128)], mc_b[:],
                                 start=True, stop=False)
                nc.tensor.matmul(ps_mc[:], ln_b[:, ts(c, 128)], onesrT[:],
                                 start=False, stop=True)
                tmp = tpool.tile([128, T], F32, tag="tmp", name="tmp")
                nc.vector.tensor_mul(tmp[:], x[:, ts(c, T)], ps_rs[:])
                nc.vector.tensor_sub(h[:, ts(c, T)], tmp[:], ps_mc[:])


        # ---- patch embed: x = convW^T @ patches + pe
        for m in range(C):
            ps = psum()
            for c in range(C):
                nc.tensor.matmul(ps[:], cw_sb[:, c * DIM + m * 128:c * DIM + m * 128 + 128],
                                 px_sb[:, ts(c, T)], start=(c == 0), stop=False)
            nc.tensor.matmul(ps[:], pe_sb[:, ts(m, 128)], ri_sb[:],
                             start=False, stop=True)
            nc.vector.tensor_copy(x[:, ts(m, T)], ps[:])

        for L in range(DEPTH):
            ln1r = lp_pool.tile([1, DIM], BF16, tag="ln1r", name="ln1r")
            nc.sync.dma_start(ln1r[:], ln1r_d[L])
            ln1b = lp_pool.tile([1, DIM], BF16, tag="ln1b", name="ln1b")
            nc.sync.dma_start(ln1b[:], ln1b_d[L])
            ln2r = lp_pool.tile([1, DIM], BF16, tag="ln2r", name="ln2r")
            nc.sync.dma_start(ln2r[:], ln2r_d[L])
            ln2b = lp_pool.tile([1, DIM], BF16, tag="ln2b", name="ln2b")
            nc.sync.dma_start(ln2b[:], ln2b_d[L])
            b1c = lp_pool.tile([128, FC], F32, tag="b1c", name="b1c")
            nc.sync.dma_start(b1c[:], b1_d[L])
            b2p = lp_pool.tile([1, DIM], BF16, tag="b2p", name="b2p")
            nc.sync.dma_start(b2p[:], b2_d[L])

            # ---- LN1
            h = hpool.tile([128, C * T], BF16, tag="h", name="h")
            layernorm(ln1r, ln1b, h)

            # ---- q, k (feature-major [768, 392])
            qw = wq_pool.tile([128, C * DIM], BF16, tag="wqkv", name="wqkv")
            nc.sync.dma_start(qw[:], qw_d[L])
            for m in range(C):
                ps = psum()
                for c in range(C):
                    nc.tensor.matmul(ps[:], qw[:, c * DIM + m * 128:c * DIM + m * 128 + 128],
                                     h[:, ts(c, T)], start=(c == 0), stop=(c == C - 1))
                nc.any.tensor_copy(qs[:, ts(m, T)], ps[:])
            kw = wq_pool.tile([128, C * DIM], BF16, tag="wqkv", name="wqkv")
            nc.sync.dma_start(kw[:], kw_d[L])
            for m in range(C):
                ps = psum()
                for c in range(C):
                    nc.tensor.matmul(ps[:], kw[:, c * DIM + m * 128:c * DIM + m * 128 + 128],
                                     h[:, ts(c, T)], start=(c == 0), stop=(c == C - 1))
                nc.any.tensor_copy(ks[:, ts(m, T)], ps[:])

            vw = wq_pool.tile([128, C * DIM], BF16, tag="wqkv", name="wqkv")
            nc.sync.dma_start(vw[:], vw_d[L])

            # ---- attention, 4-stage software pipeline over images
            vT = [None] * BL
            Etl = [None] * BL
            izl = [None] * BL
            zbl = [None] * BL

            def stage_a(b):       # vT, S^T, exp
                vT[b] = vpool.tile([KEEP, DIM], BF16, tag="vT", name="vT")
                for half in range(2):
                    psv = psum((KEEP, DIM // 2))
                    for c in range(C):
                        nc.tensor.matmul(
                            psv[:],
                            h[:, c * T + b * KEEP:c * T + b * KEEP + KEEP],
                            vw[:, c * DIM + half * 384:c * DIM + half * 384 + 384],
                            start=(c == 0), stop=(c == C - 1))
                    nc.any.tensor_copy(vT[b][:, ts(half, 384)], psv[:])
                pss = [psum((KEEP, 6 * KEEP)), psum((KEEP, 6 * KEEP))]
                for hh in range(NH):
                    j, par = hh // 2, hh % 2
                    nc.tensor.matmul(
                        pss[par][:, ts(j, KEEP)],
                        ks[64 * par:64 * par + 64, j * T + b * KEEP:j * T + b * KEEP + KEEP],
                        qs[64 * par:64 * par + 64, j * T + b * KEEP:j * T + b * KEEP + KEEP],
                        start=True, stop=True)
                E0 = epool.tile([KEEP, 6 * KEEP], BF16, tag="E", name="E")
                E1 = epool.tile([KEEP, 6 * KEEP], BF16, tag="E", name="E")
                nc.scalar.activation(E0[:], pss[0][:], AFT.Exp, bias=zer49[:], scale=0.125)
                nc.scalar.activation(E1[:], pss[1][:], AFT.Exp, bias=zer49[:], scale=0.125)
                Etl[b] = (E0, E1)

            def stage_b(b):       # Z and 1/Z (GpSimd partition reduce; PE/PSUM-free)
                iz0 = r2pool.tile([1, 6 * KEEP], BF16, tag="iz0", name="iz0")
                iz1 = r2pool.tile([1, 6 * KEEP], BF16, tag="iz1", name="iz1")
                izl[b] = (iz0, iz1)
                zbl[b] = (psum((KEEP, 6 * KEEP)), psum((KEEP, 6 * KEEP)))
                for par in range(2):
                    psu = zbl[b][par]
                    nc.tensor.matmul(psu[0:1, :], ones49[:], Etl[b][par][:],
                                     start=True, stop=True)
                    with nc.allow_low_precision(reason="bf16 softmax denominators"):
                        nc.vector.reciprocal(izl[b][par][:], psu[0:1, :])

            def stage_c(b):       # normalize E in place (reuses the Z tile)
                for par in range(2):
                    psb = zbl[b][par]
                    nc.tensor.matmul(psb[:], onesr1[:, :KEEP], izl[b][par][:],
                                     start=True, stop=True)
                    E = Etl[b][par]
                    nc.vector.tensor_mul(E[:], E[:], psb[:])

            def stage_d(b):       # O = V @ A^T, packed into feature-major aO
                pso = psum((128, 6 * KEEP))
                for j in range(C):
                    for par in range(2):
                        hh = 2 * j + par
                        nc.tensor.matmul(
                            pso[64 * par:64 * par + 64, ts(j, KEEP)],
                            vT[b][:, ts(hh, 64)],
                            Etl[b][par][:, ts(j, KEEP)],
                            start=True, stop=True)
                for j in range(C):
                    nc.any.tensor_copy(
                        aO[:, j * T + b * KEEP:j * T + b * KEEP + KEEP],
                        pso[:, ts(j, KEEP)])

            for i in range(BL + 3):
                if i < BL:
                    stage_a(i)
                if 0 <= i - 1 < BL:
                    stage_b(i - 1)
                if 0 <= i - 2 < BL:
                    stage_c(i - 2)
                if 0 <= i - 3 < BL:
                    stage_d(i - 3)

            # ---- proj + residual
            pw = wp_pool.tile([128, C * DIM], BF16, tag="wp", name="wp")
            nc.sync.dma_start(pw[:], pw_d[L])
            for m in range(C):
                ps = psum()
                for c in range(C):
                    nc.tensor.matmul(ps[:], pw[:, c * DIM + m * 128:c * DIM + m * 128 + 128],
                                     aO[:, ts(c, T)], start=(c == 0), stop=(c == C - 1))
                nc.vector.tensor_add(x[:, ts(m, T)], x[:, ts(m, T)], ps[:])

            # ---- LN2
            h2 = hpool.tile([128, C * T], BF16, tag="h", name="h")
            layernorm(ln2r, ln2b, h2)

            # ---- FFN phase 1: g = gelu(h2 @ w1 + b1), 24 chunks
            for c in range(FC):
                if c % 6 == 0:
                    w1t = w1_pool.tile([128, C * DIM], BF16, tag="w1", name="w1")
                    nc.sync.dma_start(w1t[:], w1_d[L, c // 6])
                psf = psum()
                for ci in range(C):
                    nc.tensor.matmul(
                        psf[:], w1t[:, ci * DIM + (c % 6) * 128:ci * DIM + (c % 6) * 128 + 128],
                        h2[:, ts(ci, T)], start=(ci == 0), stop=(ci == C - 1))
                nc.scalar.activation(g[:, ts(c, T)], psf[:], AFT.Gelu_apprx_tanh,
                                     bias=b1c[:, c:c + 1])

            # ---- FFN phase 2: x += g @ w2 + b2
            w2t = w2_pool.tile([128, 4 * C * DIM], BF16, tag="w2", name="w2")
            nc.sync.dma_start(w2t[:], w2_d[L])
            for m in range(C):
                ps = psum()
                nc.tensor.matmul(ps[:], b2p[:, ts(m, 128)], onesrT[:],
                                 start=True, stop=False)
                for c in range(FC):
                    nc.tensor.matmul(
                        ps[:], w2t[:, c * DIM + m * 128:c * DIM + m * 128 + 128],
                        g[:, ts(c, T)], start=False, stop=(c == FC - 1))
                nc.vector.tensor_add(x[:, ts(m, T)], x[:, ts(m, T)], ps[:])

        for c in range(C):
            nc.sync.dma_start(out_d[c * 128:(c + 1) * 128, :], x[:, ts(c, T)])
        es.close()

    nc.compile()
    return nc


def _prep_shared(inputs):
    """Host-side packing of weights (shared across cores), bf16."""
    sh = {}
    sh["convw"] = _chunk_pack(
        np.asarray(inputs["conv_w"], np.float32).reshape(P * P * 3, DIM), DIM).astype(bf16)
    qkv = np.asarray(inputs["qkv_w"], np.float32)
    sh["qw"] = np.stack([_chunk_pack(qkv[L, :, :DIM], DIM) for L in range(DEPTH)]).astype(bf16)
    sh["kw"] = np.stack([_chunk_pack(qkv[L, :, DIM:2 * DIM], DIM) for L in range(DEPTH)]).astype(bf16)
    sh["vw"] = np.stack([_chunk_pack(qkv[L, :, 2 * DIM:], DIM) for L in range(DEPTH)]).astype(bf16)
    pwt = np.asarray(inputs["proj_w"], np.float32)
    sh["pw"] = np.stack([_chunk_pack(pwt[L], DIM) for L in range(DEPTH)]).astype(bf16)
    w1 = np.asarray(inputs["w1"], np.float32)
    sh["w1q"] = np.stack([
        np.stack([_chunk_pack(w1[L][:, 768 * qt:768 * (qt + 1)], DIM) for qt in range(4)])
        for L in range(DEPTH)]).astype(bf16)
    w2 = np.asarray(inputs["w2"], np.float32)
    sh["w2q"] = np.stack([
        np.concatenate([_chunk_pack(w2[L][768 * qt:768 * (qt + 1), :], DIM)
                        for qt in range(4)], axis=1)
        for L in range(DEPTH)]).astype(bf16)
    sh["b2p"] = np.asarray(inputs["b2"], np.float32).reshape(DEPTH, 1, DIM).astype(bf16)
    sh["b1c"] = np.ascontiguousarray(
        np.asarray(inputs["b1"], np.float32).reshape(DEPTH, FC, 128).transpose(0, 2, 1))
    sh["ln1r"] = np.asarray(inputs["ln1_s"], np.float32).reshape(DEPTH, 1, DIM).astype(bf16)
    sh["ln1b"] = (-np.asarray(inputs["ln1_b"], np.float32)).reshape(DEPTH, 1, DIM).astype(bf16)
    sh["ln2r"] = np.asarray(inputs["ln2_s"], np.float32).reshape(DEPTH, 1, DIM).astype(bf16)
    sh["ln2b"] = (-np.asarray(inputs["ln2_b"], np.float32)).reshape(DEPTH, 1, DIM).astype(bf16)
    return sh


def kernel(**inputs) -> np.ndarray:
    if "nc" not in _cached:
        _cached["nc"] = _build()
    nc = _cached["nc"]

    mask = np.minimum(np.asarray(inputs["mask_idx"]).astype(np.int64), 196)
    mask = np.maximum(mask, 1)
    pidx = mask - 1
    imgs = np.asarray(inputs["inputs"], np.float32)
    pat = imgs.reshape(B, 14, P, 14, P, 3).transpose(0, 1, 3, 2, 4, 5).reshape(B, 196, P * P * 3)
    pat = pat[:, pidx]                                     # [B, 49, 768]
    pe = np.asarray(inputs["pe"], np.float32)[0, mask]     # [49, 768]

    sh = _prep_shared(inputs)
    sh["peL"] = pe.astype(bf16)
    sh["repI"] = np.tile(np.eye(KEEP, dtype=np.float32), (1, BL)).astype(bf16)

    in_maps = []
    for r in range(NCORES):
        pxT = pat[r * BL:(r + 1) * BL].reshape(T, P * P * 3).T   # [768, 392]
        m = dict(sh)
        m["pxT"] = _chunk_pack(np.ascontiguousarray(pxT), T).astype(bf16)
        in_maps.append(m)

    res = run_bass_kernel_spmd(nc, in_maps, core_ids=list(range(NCORES)),
                               trace=_cached.get("trace", False),
                               tmpdir=_cached.get("tmpdir"))
    _cached["last_result"] = res

    out = np.empty((B, KEEP, DIM), np.float32)
    for r in range(NCORES):
        o = res.results[r]["out"]                          # [768, 392]
        out[r * BL:(r + 1) * BL] = o.T.reshape(BL, KEEP, DIM)
    return out



# revision 14
# speedup vs baseline: 1.1010x; 1.1010x over previous
"""MAE ViT-Base encoder (masked, KEEP=49) on 8 TRN2 NeuronCores.

Data-parallel over batch (8 images/core). Feature-major activations
[768, 392] on-chip; fp8 DoubleRow matmuls (weights x64, activations
~unit-scale) with fp32 PSUM accumulation and an fp32 residual stream.
LayerNorm scale/bias are folded into the adjacent weight matrices on
the host, so on-chip LN is only (x - mean) * rstd with rstd computed
as exp(-0.5*ln(var+eps)) (keeps ScalarE on one LUT table-set with the
softmax exp). Softmax denominators are produced as PSUM columns
(partition-parallel reciprocal), transposed on the PE, and broadcast
by GpSimd into a full scale tile that is fused into the attention
output copy. V^T is computed two images at a time ([98, 384] PSUM
tiles) for better PE utilization.
"""

import numpy as np
import ml_dtypes

import concourse.bass as bass
import concourse.mybir as mybir
import concourse.tile as tile
from concourse import bacc
from concourse.bass import ts
from concourse.bass_utils import run_bass_kernel_spmd

AFT = mybir.ActivationFunctionType
BF16 = mybir.dt.bfloat16
F32 = mybir.dt.float32
FP8 = mybir.dt.float8e4
ALU = mybir.AluOpType
DR = mybir.MatmulPerfMode.DoubleRow

B, P, DIM, DEPTH, NH, DH, FF = 64, 16, 768, 12, 12, 64, 3072
KEEP = 49
NCORES = 8
BL = B // NCORES          # 8 images per core
T = BL * KEEP             # 392 tokens per core
Tp = 400                  # padded token stride (fp8 DR needs 16B-aligned)
C = DIM // 128            # 6 feature chunks
FC = FF // 128            # 24 ffn chunks
EPS = 1e-9

FP8_FFN = False            # fp8 DoubleRow for w1/w2 (h2, g in fp8)
FP8_ATT = False            # fp8 DoubleRow for qkv/proj (h, aO in fp8)
WS = 64.0                 # fp8 weight scale
FP8MAX = 240.0

bf16 = ml_dtypes.bfloat16
f8 = mybir.dt.np(FP8)
_cached = {}


def _chunk_pack(w, cols):
    """[768, cols] -> [128, 6*cols] with tile[p, c*cols+x] = w[c*128+p, x]."""
    return np.ascontiguousarray(
        w.reshape(C, 128, cols).transpose(1, 0, 2).reshape(128, C * cols))


def _dr_pack(w, scale):
    """[K, O] -> [128, (K/256)*2*O] fp8 for DoubleRow.

    Layout [p, d, j, o] = w[d*256 + j*128 + p, o] * scale, clipped.
    """
    K, O = w.shape
    d = K // 256
    v = np.clip(w * scale, -FP8MAX, FP8MAX)
    return np.ascontiguousarray(
        v.reshape(d, 2, 128, O).transpose(2, 0, 1, 3).reshape(128, d * 2 * O)
    ).astype(f8)


def _col_pack(v):
    """[768] -> [128, 6] with out[p, c] = v[c*128+p]."""
    return np.ascontiguousarray(v.reshape(C, 128).T)


def _build(has_bv, has_b2):
    AS = 8.0 if FP8_ATT else 1.0
    att_dt = FP8 if FP8_ATT else BF16
    ffn_dt = FP8 if FP8_FFN else BF16
    att_ws = WS if FP8_ATT else 1.0
    ffn_ws = WS if FP8_FFN else 1.0

    nc = bacc.Bacc("TRN2", target_bir_lowering=False, debug=False,
                   enable_asserts=False, num_devices=NCORES)

    def din(name, shape, dt=BF16):
        return nc.dram_tensor(name, shape, dt, kind="ExternalInput").ap()

    pxT = din("pxT", [128, C * T])
    convw = din("convw", [128, C * DIM])
    peL = din("peL", [KEEP, DIM])
    repI = din("repI", [KEEP, T])
    eye_d = din("eye", [128, 128])
    if FP8_ATT:
        qw_d = din("qw", [DEPTH, 128, 6 * DIM], FP8)
        kw_d = din("kw", [DEPTH, 128, 6 * DIM], FP8)
        vw_d = din("vw", [DEPTH, 128, 6 * DIM], FP8)
        pw_d = din("pw", [DEPTH, 128, 6 * DIM], FP8)
    else:
        qw_d = din("qw", [DEPTH, 128, C * DIM])
        kw_d = din("kw", [DEPTH, 128, C * DIM])
        vw_d = din("vw", [DEPTH, 128, C * DIM])
        pw_d = din("pw", [DEPTH, 128, C * DIM])
    if FP8_FFN:
        w1_d = din("w1q", [DEPTH, 128, 6 * FF], FP8)
        w2_d = din("w2q", [DEPTH, 128, 24 * DIM], FP8)
    else:
        w1_d = din("w1q", [DEPTH, 4, 128, C * DIM])
        w2_d = din("w2q", [DEPTH, 128, 4 * C * DIM])
    b1_d = din("b1c", [DEPTH, 128, FC], F32)
    bqk_d = din("bqk", [DEPTH, 128, 2 * C], F32)
    if has_bv:
        bv_d = din("bvr", [DEPTH, 1, DIM])
    if has_b2:
        b2_d = din("b2p", [DEPTH, 1, DIM])
    out_d = nc.dram_tensor("out", [DIM, T], F32, kind="ExternalOutput").ap()

    CT = C * T
    exp_scale = 0.125 / (att_ws * att_ws)

    with tile.TileContext(nc) as tc:
        from contextlib import ExitStack
        es = ExitStack()
        cpool = es.enter_context(tc.tile_pool(name="consts", bufs=1))
        apool = es.enter_context(tc.tile_pool(name="acts", bufs=1))
        hpool = es.enter_context(tc.tile_pool(name="h", bufs=2))
        vpool = es.enter_context(tc.tile_pool(name="vt", bufs=9))
        wq_pool = es.enter_context(tc.tile_pool(name="wqkv", bufs=2))
        w1_pool = es.enter_context(tc.tile_pool(name="w1", bufs=2))
        w2_pool = es.enter_context(tc.tile_pool(name="w2", bufs=1))
        lp_pool = es.enter_context(tc.tile_pool(name="lparam", bufs=2))
        tpool = es.enter_context(tc.tile_pool(name="tmp", bufs=3))
        rpool = es.enter_context(tc.tile_pool(name="rows", bufs=2))
        bpool = es.enter_context(tc.tile_pool(name="bcast", bufs=4))
        epool = es.enter_context(tc.tile_pool(name="etiles", bufs=16))
        zpool = es.enter_context(tc.tile_pool(name="ztiles", bufs=4))
        pspool = es.enter_context(tc.tile_pool(name="ps", bufs=8, space="PSUM"))

        def psum(shape=(128, T), dt=F32):
            return pspool.tile(list(shape), dt, tag="ps", name="ps")

        # ---- small constants
        ones128 = cpool.tile([128, 1], BF16, tag="ones128", name="ones128")
        nc.vector.memset(ones128[:], 1.0)
        ones49 = cpool.tile([KEEP, 1], BF16, tag="ones49", name="ones49")
        nc.vector.memset(ones49[:], 1.0)
        onesrT = cpool.tile([1, T], BF16, tag="onesrT", name="onesrT")
        nc.vector.memset(onesrT[:], 1.0)
        ones98r = cpool.tile([1, 2 * KEEP], BF16, tag="ones98r", name="ones98r")
        nc.vector.memset(ones98r[:], 1.0)
        eps1 = cpool.tile([1, 1], F32, tag="eps1", name="eps1")
        nc.vector.memset(eps1[:], EPS)
        eye = cpool.tile([128, 128], BF16, tag="eye", name="eye")
        nc.sync.dma_start(eye[:], eye_d)

        # ---- patch-embed constants share the w2 slot (freed before L0 FFN2)
        NPX, NCW, NPE, NRI = CT, C * DIM, DIM, T
        W2COLS = 24 * DIM if FP8_FFN else 4 * C * DIM
        cst = w2_pool.tile([128, NPX + NCW + NPE + NRI], BF16,
                           tag="w2", name="w2")
        px_sb = cst[:, 0:NPX]
        cw_sb = cst[:, NPX:NPX + NCW]
        pe_sb = cst[0:KEEP, NPX + NCW:NPX + NCW + NPE]
        ri_sb = cst[0:KEEP, NPX + NCW + NPE:NPX + NCW + NPE + NRI]
        nc.sync.dma_start(px_sb, pxT)
        nc.sync.dma_start(cw_sb, convw)
        nc.sync.dma_start(pe_sb, peL)
        nc.sync.dma_start(ri_sb, repI)

        x = apool.tile([128, CT], F32, tag="x", name="x")
        xb = apool.tile([128, CT], BF16, tag="xb", name="xb")
        x2 = apool.tile([128, CT], BF16, tag="x2", name="x2")
        qs = apool.tile([128, CT], BF16, tag="qs", name="qs")
        ks = apool.tile([128, CT], BF16, tag="ks", name="ks")
        aO = apool.tile([128, C * Tp], att_dt, tag="aO", name="aO")
        g = apool.tile([128, FC * Tp], ffn_dt, tag="g", name="g")

        def pair_ap(t, d, lo, n):
            """[128, 2, n] AP over chunk pair d of a Tp-padded tile."""
            sl = t[:, 2 * d * Tp:(2 * d + 2) * Tp]
            return sl.rearrange("p (j t) -> p j t", j=2)[:, :, lo:lo + n]

        def layernorm(h, out_dt):
            ps_sx = psum((1, T))
            ps_sxx = psum((1, T))
            for c in range(C):
                nc.gpsimd.tensor_copy(xb[:, ts(c, T)], x[:, ts(c, T)])
                nc.vector.tensor_mul(x2[:, ts(c, T)], xb[:, ts(c, T)],
                                     xb[:, ts(c, T)])
                nc.tensor.matmul(ps_sx[:], ones128[:], xb[:, ts(c, T)],
                                 start=(c == 0), stop=(c == C - 1))
                nc.tensor.matmul(ps_sxx[:], ones128[:], x2[:, ts(c, T)],
                                 start=(c == 0), stop=(c == C - 1))
            m_row = rpool.tile([1, T], BF16, tag="m_row", name="m_row")
            with nc.allow_low_precision(reason="bf16 LN mean"):
                nc.scalar.mul(m_row[:], ps_sx[:], 1.0 / DIM)
            msq = rpool.tile([1, T], F32, tag="msq", name="msq")
            nc.scalar.activation(msq[:], ps_sx[:], AFT.Square, scale=1.0 / DIM)
            var = rpool.tile([1, T], F32, tag="var", name="var")
            nc.vector.scalar_tensor_tensor(var[:], ps_sxx[:], 1.0 / DIM,
                                           msq[:], ALU.mult, ALU.subtract)
            lnv = rpool.tile([1, T], F32, tag="lnv", name="lnv")
            nc.scalar.activation(lnv[:], var[:], AFT.Ln, bias=eps1[:])
            rstd = rpool.tile([1, T], BF16, tag="rstd", name="rstd")
            with nc.allow_low_precision(reason="bf16 LN rstd"):
                nc.scalar.activation(rstd[:], lnv[:], AFT.Exp, scale=-0.5)
            Rb = bpool.tile([128, T], BF16, tag="Rb", name="Rb")
            nc.gpsimd.partition_broadcast(Rb[:], rstd[:])
            Mb = bpool.tile([128, T], BF16, tag="Mb", name="Mb")
            nc.gpsimd.partition_broadcast(Mb[:], m_row[:])
            for c in range(C):
                tmp = tpool.tile([128, T], BF16, tag="tmp", name="tmp")
                nc.vector.tensor_sub(tmp[:], xb[:, ts(c, T)], Mb[:])
                with nc.allow_low_precision(reason="narrow LN output"):
                    nc.vector.tensor_mul(h[:, c * Tp:c * Tp + T], tmp[:], Rb[:])

        # ---- patch embed: x = convW^T @ patches + pe
        for m in range(C):
            ps = psum()
            for c in range(C):
                nc.tensor.matmul(
                    ps[:], cw_sb[:, c * DIM + m * 128:c * DIM + m * 128 + 128],
                    px_sb[:, ts(c, T)], start=(c == 0), stop=False)
            nc.tensor.matmul(ps[:], pe_sb[:, ts(m, 128)], ri_sb[:],
                             start=False, stop=True)
            nc.vector.tensor_copy(x[:, ts(m, T)], ps[:])

        for L in range(DEPTH):
            b1c = lp_pool.tile([128, FC], F32, tag="b1c", name="b1c")
            nc.sync.dma_start(b1c[:], b1_d[L])
            bqk = lp_pool.tile([128, 2 * C], F32, tag="bqk", name="bqk")
            nc.sync.dma_start(bqk[:], bqk_d[L])
            if has_bv:
                bvr = lp_pool.tile([1, DIM], BF16, tag="bvr", name="bvr")
                nc.sync.dma_start(bvr[:], bv_d[L])
            if has_b2:
                b2p = lp_pool.tile([1, DIM], BF16, tag="b2p", name="b2p")
                nc.sync.dma_start(b2p[:], b2_d[L])

            # ---- LN1 (scale/bias folded into qkv weights host-side)
            h = hpool.tile([128, C * Tp], att_dt, tag="h", name="h")
            layernorm(h, att_dt)

            # ---- q, k (feature-major [768, 392], kept at ws scale)
            qw = wq_pool.tile([128, 6 * DIM] if FP8_ATT else [128, C * DIM],
                              att_dt, tag="wqkv", name="wqkv")
            nc.sync.dma_start(qw[:], qw_d[L])
            kw = wq_pool.tile([128, 6 * DIM] if FP8_ATT else [128, C * DIM],
                              att_dt, tag="wqkv", name="wqkv")
            nc.sync.dma_start(kw[:], kw_d[L])

            def qk_mm(dst, w, bcol):
                for m in range(C):
                    ps = psum()
                    if FP8_ATT:
                        for d in range(3):
                            wp = w[:, d * 2 * DIM:(d + 1) * 2 * DIM].rearrange(
                                "p (j o) -> p j o", j=2)[:, :, ts(m, 128)]
                            nc.tensor.matmul(ps[:], wp, pair_ap(h, d, 0, T),
                                             start=(d == 0), stop=(d == 2),
                                             perf_mode=DR)
                    else:
                        for c in range(C):
                            nc.tensor.matmul(
                                ps[:],
                                w[:, c * DIM + m * 128:c * DIM + m * 128 + 128],
                                h[:, c * Tp:c * Tp + T],
                                start=(c == 0), stop=(c == C - 1))
                    with nc.allow_low_precision(reason="bf16 q/k"):
                        nc.scalar.activation(dst[:, ts(m, T)], ps[:],
                                             AFT.Identity, bias=bcol[:, m:m + 1])
            qk_mm(qs, qw, bqk[:, 0:C])
            qk_mm(ks, kw, bqk[:, C:2 * C])

            vw = wq_pool.tile([128, 6 * DIM] if FP8_ATT else [128, C * DIM],
                              att_dt, tag="wqkv", name="wqkv")
            nc.sync.dma_start(vw[:], vw_d[L])

            # ---- vT per image: psv [49, 384] per half
            vT = [None] * BL
            for b in range(BL):
                vT[b] = vpool.tile([KEEP, DIM], BF16, tag="vT", name="vT")
                for half in range(2):
                    psv = psum((KEEP, DIM // 2))
                    if FP8_ATT:
                        for d in range(3):
                            nc.tensor.matmul(
                                psv[:], pair_ap(h, d, b * KEEP, KEEP),
                                vw[:, d * 2 * DIM:(d + 1) * 2 * DIM].rearrange(
                                    "p (j o) -> p j o", j=2)[:, :, ts(half, 384)],
                                start=(d == 0), stop=False if has_bv else (d == 2),
                                perf_mode=DR)
                    else:
                        for c in range(C):
                            nc.tensor.matmul(
                                psv[:],
                                h[:, c * Tp + b * KEEP:c * Tp + b * KEEP + KEEP],
                                vw[:, c * DIM + half * 384:c * DIM + half * 384 + 384],
                                start=(c == 0), stop=False if has_bv else (c == C - 1))
                    if has_bv:
                        nc.tensor.matmul(psv[:], ones98r[:, 0:KEEP],
                                         bvr[:, ts(half, 384)],
                                         start=False, stop=True)
                    with nc.allow_low_precision(reason="bf16 vT"):
                        nc.scalar.activation(vT[b][:, ts(half, 384)], psv[:],
                                             AFT.Identity,
                                             scale=AS / att_ws)

            # ---- attention: scores + exp + Z cols + iz transpose
            Etl = [None] * BL
            izr = [None] * BL
            W98 = 2 * KEEP

            def stage_scores(b):
                pss = [psum((KEEP, 6 * KEEP)), psum((KEEP, 6 * KEEP))]
                for hh in range(NH):
                    j, par = hh // 2, hh % 2
                    nc.tensor.matmul(
                        pss[par][:, ts(j, KEEP)],
                        ks[64 * par:64 * par + 64,
                           j * T + b * KEEP:j * T + b * KEEP + KEEP],
                        qs[64 * par:64 * par + 64,
                           j * T + b * KEEP:j * T + b * KEEP + KEEP],
                        start=True, stop=True)
                E0 = epool.tile([KEEP, 6 * KEEP], BF16, tag="E", name="E")
                E1 = epool.tile([KEEP, 6 * KEEP], BF16, tag="E", name="E")
                nc.scalar.activation(E0[:], pss[0][:], AFT.Exp, scale=exp_scale)
                nc.scalar.activation(E1[:], pss[1][:], AFT.Exp, scale=exp_scale)
                Etl[b] = (E0, E1)

            def stage_z(b):
                zc = psum((128, 8))
                for par in range(2):
                    for c3 in range(3):
                        nc.tensor.matmul(
                            zc[0:W98, par * 3 + c3:par * 3 + c3 + 1],
                            Etl[b][par][:, c3 * W98:(c3 + 1) * W98],
                            ones49[:], start=True, stop=True)
                izc = zpool.tile([128, 8], BF16, tag="izc", name="izc")
                with nc.allow_low_precision(reason="bf16 softmax recip"):
                    nc.vector.reciprocal(izc[0:W98, 0:6], zc[0:W98, 0:6])
                pst = psum((1, 6 * W98), BF16)
                for i in range(6):
                    nc.tensor.transpose(pst[0:1, i * W98:(i + 1) * W98],
                                        izc[0:W98, i:i + 1], eye[0:W98, 0:W98])
                izr[b] = zpool.tile([1, 6 * W98], BF16, tag="izT", name="izT")
                nc.vector.tensor_copy(izr[b][:], pst[:])

            for b in range(BL):
                stage_scores(b)
                if b >= 1:
                    stage_z(b - 1)
            stage_z(BL - 1)

            # ---- AV with fused 1/Z scaling into aO
            for b in range(BL):
                izb = bpool.tile([128, 6 * W98], BF16, tag="izb", name="izb")
                nc.gpsimd.partition_broadcast(izb[:], izr[b][:])
                pso = psum((128, 6 * KEEP))
                for j in range(C):
                    for par in range(2):
                        hh = 2 * j + par
                        nc.tensor.matmul(
                            pso[64 * par:64 * par + 64, ts(j, KEEP)],
                            vT[b][:, ts(hh, 64)],
                            Etl[b][par][:, ts(j, KEEP)],
                            start=True, stop=True)
                a3 = aO[:].rearrange("p (c t) -> p c t", c=C)[:, :, b * KEEP:
                                                             (b + 1) * KEEP]
                p3 = pso[:].rearrange("p (c t) -> p c t", c=C)
                with nc.allow_low_precision(reason="narrow attention out"):
                    nc.vector.tensor_tensor(
                        a3[0:64], p3[0:64],
                        izb[0:64, 0:294].rearrange("p (c t) -> p c t", c=C),
                        ALU.mult)
                    nc.vector.tensor_tensor(
                        a3[64:128], p3[64:128],
                        izb[64:128, 294:588].rearrange("p (c t) -> p c t", c=C),
                        ALU.mult)

            # ---- proj + residual
            pw = wq_pool.tile([128, 6 * DIM] if FP8_ATT else [128, C * DIM],
                              att_dt, tag="wp", name="wp")
            nc.sync.dma_start(pw[:], pw_d[L])
            for m in range(C):
                ps = psum()
                if FP8_ATT:
                    for d in range(3):
                        nc.tensor.matmul(
                            ps[:],
                            pw[:, d * 2 * DIM:(d + 1) * 2 * DIM].rearrange(
                                "p (j o) -> p j o", j=2)[:, :, ts(m, 128)],
                            pair_ap(aO, d, 0, T),
                            start=(d == 0), stop=(d == 2), perf_mode=DR)
                else:
                    for c in range(C):
                        nc.tensor.matmul(
                            ps[:],
                            pw[:, c * DIM + m * 128:c * DIM + m * 128 + 128],
                            aO[:, c * Tp:c * Tp + T],
                            start=(c == 0), stop=(c == C - 1))
                nc.vector.scalar_tensor_tensor(
                    x[:, ts(m, T)], ps[:], 1.0 / (att_ws * AS),
                    x[:, ts(m, T)], ALU.mult, ALU.add)

            # ---- LN2 (scale/bias folded into w1/b1)
            h2 = hpool.tile([128, C * Tp], ffn_dt, tag="h", name="h")
            layernorm(h2, ffn_dt)

            # ---- FFN phase 1: g = gelu(h2 @ w1 + b1)
            if FP8_FFN:
                w1t = w1_pool.tile([128, 6 * FF], FP8, tag="w1", name="w1")
                nc.sync.dma_start(w1t[:], w1_d[L])
            for c in range(FC):
                if not FP8_FFN and c % 6 == 0:
                    w1t = w1_pool.tile([128, C * DIM], BF16, tag="w1", name="w1")
                    nc.sync.dma_start(w1t[:], w1_d[L, c // 6])
                psf = psum()
                if FP8_FFN:
                    for d in range(3):
                        nc.tensor.matmul(
                            psf[:],
                            w1t[:, d * 2 * FF:(d + 1) * 2 * FF].rearrange(
                                "p (j o) -> p j o", j=2)[:, :, ts(c, 128)],
                            pair_ap(h2, d, 0, T),
                            start=(d == 0), stop=(d == 2), perf_mode=DR)
                else:
                    for ci in range(C):
                        nc.tensor.matmul(
                            psf[:],
                            w1t[:, ci * DIM + (c % 6) * 128:
                                ci * DIM + (c % 6) * 128 + 128],
                            h2[:, ci * Tp:ci * Tp + T],
                            start=(ci == 0), stop=(ci == C - 1))
                with nc.allow_low_precision(reason="narrow gelu out"):
                    nc.scalar.activation(g[:, c * Tp:c * Tp + T], psf[:],
                                         AFT.Gelu_apprx_tanh,
                                         bias=b1c[:, c:c + 1], scale=1.0 / ffn_ws)

            # ---- FFN phase 2: x += g @ w2 + b2
            w2t = w2_pool.tile([128, W2COLS], ffn_dt, tag="w2", name="w2")
            nc.sync.dma_start(w2t[:], w2_d[L])
            for m in range(C):
                ps = psum()
                first = True
                if has_b2:
                    nc.tensor.matmul(ps[:], b2p[:, ts(m, 128)], onesrT[:],
                                     start=True, stop=False)
                    first = False
                if FP8_FFN:
                    for d in range(12):
                        nc.tensor.matmul(
                            ps[:],
                            w2t[:, d * 2 * DIM:(d + 1) * 2 * DIM].rearrange(
                                "p (j o) -> p j o", j=2)[:, :, ts(m, 128)],
                            pair_ap(g, d, 0, T),
                            start=first and (d == 0), stop=(d == 11),
                            perf_mode=DR)
                else:
                    for c in range(FC):
                        nc.tensor.matmul(
                            ps[:],
                            w2t[:, c * DIM + m * 128:c * DIM + m * 128 + 128],
                            g[:, c * Tp:c * Tp + T],
                            start=first and (c == 0), stop=(c == FC - 1))
                nc.vector.scalar_tensor_tensor(
                    x[:, ts(m, T)], ps[:], 1.0 / ffn_ws,
                    x[:, ts(m, T)], ALU.mult, ALU.add)

        for c in range(C):
            nc.sync.dma_start(out_d[c * 128:(c + 1) * 128, :], x[:, ts(c, T)])
        es.close()

    nc.compile()
    return nc


def _prep_shared(inputs):
    """Host-side packing of weights (shared across cores)."""
    sh = {}
    sh["convw"] = _chunk_pack(
        np.asarray(inputs["conv_w"], np.float32).reshape(P * P * 3, DIM),
        DIM).astype(bf16)
    sh["eye"] = np.eye(128, dtype=np.float32).astype(bf16)

    qkv = np.asarray(inputs["qkv_w"], np.float32)
    ln1s = np.asarray(inputs["ln1_s"], np.float32)
    ln1b = np.asarray(inputs["ln1_b"], np.float32)
    ln2s = np.asarray(inputs["ln2_s"], np.float32)
    ln2b = np.asarray(inputs["ln2_b"], np.float32)
    w1 = np.asarray(inputs["w1"], np.float32)
    b1 = np.asarray(inputs["b1"], np.float32)
    w2 = np.asarray(inputs["w2"], np.float32)
    b2 = np.asarray(inputs["b2"], np.float32)
    pwt = np.asarray(inputs["proj_w"], np.float32)

    att_ws = WS if FP8_ATT else 1.0
    ffn_ws = WS if FP8_FFN else 1.0

    qw, kw, vw, pw, w1q, w2q, b1c, bqk, bvr = [], [], [], [], [], [], [], [], []
    for L in range(DEPTH):
        Wq = qkv[L, :, :DIM] * ln1s[L][:, None]
        Wk = qkv[L, :, DIM:2 * DIM] * ln1s[L][:, None]
        Wv = qkv[L, :, 2 * DIM:] * ln1s[L][:, None]
        if FP8_ATT:
            qw.append(_dr_pack(Wq, WS))
            kw.append(_dr_pack(Wk, WS))
            vw.append(_dr_pack(Wv, WS))
            pw.append(_dr_pack(pwt[L], WS))
        else:
            qw.append(_chunk_pack(Wq, DIM).astype(bf16))
            kw.append(_chunk_pack(Wk, DIM).astype(bf16))
            vw.append(_chunk_pack(Wv, DIM).astype(bf16))
            pw.append(_chunk_pack(pwt[L], DIM).astype(bf16))
        bq = ln1b[L] @ qkv[L, :, :DIM]
        bk = ln1b[L] @ qkv[L, :, DIM:2 * DIM]
        bqk.append(np.concatenate(
            [_col_pack(bq * att_ws), _col_pack(bk * att_ws)], axis=1))
        bvr.append((ln1b[L] @ qkv[L, :, 2 * DIM:]) * att_ws)

        W1 = w1[L] * ln2s[L][:, None]
        if FP8_FFN:
            w1q.append(_dr_pack(W1, WS))
            w2q.append(_dr_pack(w2[L], WS))
        else:
            w1q.append(np.stack(
                [_chunk_pack(W1[:, 768 * qt:768 * (qt + 1)], DIM)
                 for qt in range(4)]).astype(bf16))
            w2q.append(np.concatenate(
                [_chunk_pack(w2[L][768 * qt:768 * (qt + 1), :], DIM)
                 for qt in range(4)], axis=1).astype(bf16))
        b1c.append(np.ascontiguousarray(
            (b1[L] + ln2b[L] @ w1[L]).reshape(FC, 128).T))

    sh["qw"] = np.stack(qw)
    sh["kw"] = np.stack(kw)
    sh["vw"] = np.stack(vw)
    sh["pw"] = np.stack(pw)
    sh["w1q"] = np.stack(w1q)
    sh["w2q"] = np.stack(w2q)
    sh["b1c"] = np.stack(b1c).astype(np.float32)
    sh["bqk"] = np.stack(bqk).astype(np.float32)

    bvr = np.stack(bvr)
    has_bv = bool(np.any(bvr != 0.0))
    if has_bv:
        sh["bvr"] = bvr.reshape(DEPTH, 1, DIM).astype(bf16)
    has_b2 = bool(np.any(b2 != 0.0))
    if has_b2:
        sh["b2p"] = (b2 * ffn_ws).reshape(DEPTH, 1, DIM).astype(bf16)
    return sh, has_bv, has_b2


def kernel(**inputs) -> np.ndarray:
    sh, has_bv, has_b2 = _prep_shared(inputs)
    key = ("nc", FP8_FFN, FP8_ATT, has_bv, has_b2)
    if key not in _cached:
        _cached[key] = _build(has_bv, has_b2)
    nc = _cached[key]

    mask = np.minimum(np.asarray(inputs["mask_idx"]).astype(np.int64), 196)
    mask = np.maximum(mask, 1)
    pidx = mask - 1
    imgs = np.asarray(inputs["inputs"], np.float32)
    pat = imgs.reshape(B, 14, P, 14, P, 3).transpose(0, 1, 3, 2, 4, 5)
    pat = pat.reshape(B, 196, P * P * 3)[:, pidx]          # [B, 49, 768]
    pe = np.asarray(inputs["pe"], np.float32)[0, mask]     # [49, 768]

    sh["peL"] = pe.astype(bf16)
    sh["repI"] = np.tile(np.eye(KEEP, dtype=np.float32), (1, BL)).astype(bf16)

    in_maps = []
    for r in range(NCORES):
        pxT = pat[r * BL:(r + 1) * BL].reshape(T, P * P * 3).T   # [768, 392]
        m = dict(sh)
        m["pxT"] = _chunk_pack(np.ascontiguousarray(pxT), T).astype(bf16)
        in_maps.append(m)

    res = run_bass_kernel_spmd(nc, in_maps, core_ids=list(range(NCORES)),
                               trace=_cached.get("trace", False),
                               tmpdir=_cached.get("tmpdir"))
    _cached["last_result"] = res

    out = np.empty((B, KEEP, DIM), np.float32)
    for r in range(NCORES):
        o = res.results[r]["out"]                          # [768, 392]
        out[r * BL:(r + 1) * BL] = o.T.reshape(BL, KEEP, DIM)
    return out


# revision 31
# speedup vs baseline: 1.1943x; 1.0847x over previous
"""MAE ViT-Base encoder (masked, KEEP=49) on 8 TRN2 NeuronCores.

Data-parallel over batch (8 images/core). Feature-major activations
[768, 392] on-chip; fp8 DoubleRow matmuls (weights x64, activations
~unit-scale) with fp32 PSUM accumulation and an fp32 residual stream.
LayerNorm scale/bias are folded into the adjacent weight matrices on
the host, so on-chip LN is only (x - mean) * rstd with rstd computed
as exp(-0.5*ln(var+eps)) (keeps ScalarE on one LUT table-set with the
softmax exp). Softmax denominators are produced as PSUM columns
(partition-parallel reciprocal), transposed on the PE, and broadcast
by GpSimd into a full scale tile that is fused into the attention
output copy. V^T is computed two images at a time ([98, 384] PSUM
tiles) for better PE utilization.
"""

import numpy as np
import ml_dtypes

import concourse.bass as bass
import concourse.mybir as mybir
import concourse.tile as tile
from concourse import bacc
from concourse.bass import ts
from concourse.bass_utils import run_bass_kernel_spmd

AFT = mybir.ActivationFunctionType
BF16 = mybir.dt.bfloat16
F32 = mybir.dt.float32
FP8 = mybir.dt.float8e4
ALU = mybir.AluOpType
DR = mybir.MatmulPerfMode.DoubleRow

B, P, DIM, DEPTH, NH, DH, FF = 64, 16, 768, 12, 12, 64, 3072
KEEP = 49
NCORES = 8
BL = B // NCORES          # 8 images per core
T = BL * KEEP             # 392 tokens per core
Tp = 400                  # padded token stride (fp8 DR needs 16B-aligned)
C = DIM // 128            # 6 feature chunks
FC = FF // 128            # 24 ffn chunks
EPS = 1e-9

FP8_FFN = False            # fp8 DoubleRow for w1/w2 (h2, g in fp8)
FP8_ATT = False            # fp8 DoubleRow for qkv/proj (h, aO in fp8)
WS = 64.0                 # fp8 weight scale
FP8MAX = 240.0

bf16 = ml_dtypes.bfloat16
f8 = mybir.dt.np(FP8)
_cached = {}


def _chunk_pack(w, cols):
    """[768, cols] -> [128, 6*cols] with tile[p, c*cols+x] = w[c*128+p, x]."""
    return np.ascontiguousarray(
        w.reshape(C, 128, cols).transpose(1, 0, 2).reshape(128, C * cols))


def _dr_pack(w, scale):
    """[K, O] -> [128, (K/256)*2*O] fp8 for DoubleRow.

    Layout [p, d, j, o] = w[d*256 + j*128 + p, o] * scale, clipped.
    """
    K, O = w.shape
    d = K // 256
    v = np.clip(w * scale, -FP8MAX, FP8MAX)
    return np.ascontiguousarray(
        v.reshape(d, 2, 128, O).transpose(2, 0, 1, 3).reshape(128, d * 2 * O)
    ).astype(f8)


def _col_pack(v):
    """[768] -> [128, 6] with out[p, c] = v[c*128+p]."""
    return np.ascontiguousarray(v.reshape(C, 128).T)


def _build(has_bv, has_b2):
    AS = 8.0 if FP8_ATT else 1.0
    att_dt = FP8 if FP8_ATT else BF16
    ffn_dt = FP8 if FP8_FFN else BF16
    att_ws = WS if FP8_ATT else 1.0
    ffn_ws = WS if FP8_FFN else 1.0

    nc = bacc.Bacc("TRN2", target_bir_lowering=False, debug=False,
                   enable_asserts=False, num_devices=NCORES)

    def din(name, shape, dt=BF16):
        return nc.dram_tensor(name, shape, dt, kind="ExternalInput").ap()

    pxT = din("pxT", [128, C * T])
    convw = din("convw", [128, C * DIM])
    peL = din("peL", [KEEP, DIM])
    repI = din("repI", [KEEP, T])
    eye_d = din("eye", [128, 128])
    if FP8_ATT:
        qw_d = din("qw", [DEPTH, 128, 6 * DIM], FP8)
        kw_d = din("kw", [DEPTH, 128, 6 * DIM], FP8)
        vw_d = din("vw", [DEPTH, 128, 6 * DIM], FP8)
        pw_d = din("pw", [DEPTH, 128, 6 * DIM], FP8)
    else:
        qw_d = din("qw", [DEPTH, 128, C * DIM])
        kw_d = din("kw", [DEPTH, 128, C * DIM])
        vw_d = din("vw", [DEPTH, 128, C * DIM])
        pw_d = din("pw", [DEPTH, 128, C * DIM])
    if FP8_FFN:
        w1_d = din("w1q", [DEPTH, 128, 6 * FF], FP8)
        w2_d = din("w2q", [DEPTH, 128, 24 * DIM], FP8)
    else:
        w1_d = din("w1q", [DEPTH, 4, 128, C * DIM])
        w2_d = din("w2q", [DEPTH, 128, 4 * C * DIM])
    b1_d = din("b1c", [DEPTH, 128, FC], F32)
    bqk_d = din("bqk", [DEPTH, 128, 2 * C], F32)
    if has_bv:
        bv_d = din("bvr", [DEPTH, 1, DIM])
    if has_b2:
        b2_d = din("b2p", [DEPTH, 1, DIM])
    out_d = nc.dram_tensor("out", [DIM, T], F32, kind="ExternalOutput").ap()

    CT = C * T
    exp_scale = 0.125 / (att_ws * att_ws)

    with tile.TileContext(nc) as tc:
        from contextlib import ExitStack
        es = ExitStack()
        cpool = es.enter_context(tc.tile_pool(name="consts", bufs=1))
        apool = es.enter_context(tc.tile_pool(name="acts", bufs=1))
        hpool = es.enter_context(tc.tile_pool(name="h", bufs=2))
        vpool = es.enter_context(tc.tile_pool(name="vt", bufs=9))
        wq_pool = es.enter_context(tc.tile_pool(name="wqkv", bufs=2))
        w1_pool = es.enter_context(tc.tile_pool(name="w1", bufs=2))
        w2_pool = es.enter_context(tc.tile_pool(name="w2", bufs=1))
        lp_pool = es.enter_context(tc.tile_pool(name="lparam", bufs=2))
        tpool = es.enter_context(tc.tile_pool(name="tmp", bufs=3))
        rpool = es.enter_context(tc.tile_pool(name="rows", bufs=2))
        bpool = es.enter_context(tc.tile_pool(name="bcast", bufs=4))
        epool = es.enter_context(tc.tile_pool(name="etiles", bufs=16))
        zpool = es.enter_context(tc.tile_pool(name="ztiles", bufs=4))
        pspool = es.enter_context(tc.tile_pool(name="ps", bufs=8, space="PSUM"))

        def psum(shape=(128, T), dt=F32):
            return pspool.tile(list(shape), dt, tag="ps", name="ps")

        # ---- small constants
        ones128 = cpool.tile([128, 1], BF16, tag="ones128", name="ones128")
        nc.vector.memset(ones128[:], 1.0)
        ones49 = cpool.tile([KEEP, 1], BF16, tag="ones49", name="ones49")
        nc.vector.memset(ones49[:], 1.0)
        onesrT = cpool.tile([1, T], BF16, tag="onesrT", name="onesrT")
        nc.vector.memset(onesrT[:], 1.0)
        ones98r = cpool.tile([1, 2 * KEEP], BF16, tag="ones98r", name="ones98r")
        nc.vector.memset(ones98r[:], 1.0)
        onesr1 = cpool.tile([1, 128], BF16, tag="onesr1", name="onesr1")
        nc.vector.memset(onesr1[:], 1.0)
        eps1 = cpool.tile([1, 1], F32, tag="eps1", name="eps1")
        nc.vector.memset(eps1[:], EPS)
        eye = cpool.tile([128, 128], BF16, tag="eye", name="eye")
        nc.sync.dma_start(eye[:], eye_d)

        # ---- patch-embed constants share the w2 slot (freed before L0 FFN2)
        NPX, NCW, NPE, NRI = CT, C * DIM, DIM, T
        W2COLS = 24 * DIM if FP8_FFN else 4 * C * DIM
        cst = w2_pool.tile([128, NPX + NCW + NPE + NRI], BF16,
                           tag="w2", name="w2")
        px_sb = cst[:, 0:NPX]
        cw_sb = cst[:, NPX:NPX + NCW]
        pe_sb = cst[0:KEEP, NPX + NCW:NPX + NCW + NPE]
        ri_sb = cst[0:KEEP, NPX + NCW + NPE:NPX + NCW + NPE + NRI]
        nc.sync.dma_start(px_sb, pxT)
        nc.sync.dma_start(cw_sb, convw)
        nc.sync.dma_start(pe_sb, peL)
        nc.sync.dma_start(ri_sb, repI)

        x = apool.tile([128, CT], F32, tag="x", name="x")
        xb = apool.tile([128, CT], BF16, tag="xb", name="xb")
        x2 = apool.tile([128, CT], BF16, tag="x2", name="x2")
        qs = apool.tile([128, CT], BF16, tag="qs", name="qs")
        ks = apool.tile([128, CT], BF16, tag="ks", name="ks")
        aO = apool.tile([128, C * Tp], att_dt, tag="aO", name="aO")
        g = apool.tile([128, FC * Tp], ffn_dt, tag="g", name="g")

        def pair_ap(t, d, lo, n):
            """[128, 2, n] AP over chunk pair d of a Tp-padded tile."""
            sl = t[:, 2 * d * Tp:(2 * d + 2) * Tp]
            return sl.rearrange("p (j t) -> p j t", j=2)[:, :, lo:lo + n]

        def preload(func, anchor):
            """Tiny activation to pull a LUT table-set load off the LN
            critical chain. `anchor` is a [1,1] slice of a tile written in
            the phase this load should overlap with — the data dependency
            pins the scheduler's placement."""
            scr = rpool.tile([1, 1], F32, tag="scr", name="scr")
            nc.scalar.activation(scr[:], anchor, func)

        def layernorm(h, out_dt):
            ps_sx = psum((1, T))
            ps_sxx = psum((1, T))
            for c in range(C):
                nc.vector.tensor_copy(xb[:, ts(c, T)], x[:, ts(c, T)])
                nc.vector.tensor_mul(x2[:, ts(c, T)], xb[:, ts(c, T)],
                                     xb[:, ts(c, T)])
                nc.tensor.matmul(ps_sx[:], ones128[:], xb[:, ts(c, T)],
                                 start=(c == 0), stop=(c == C - 1))
                nc.tensor.matmul(ps_sxx[:], ones128[:], x2[:, ts(c, T)],
                                 start=(c == 0), stop=(c == C - 1))
            m_row = rpool.tile([1, T], BF16, tag="m_row", name="m_row")
            with nc.allow_low_precision(reason="bf16 LN mean"):
                nc.scalar.mul(m_row[:], ps_sx[:], 1.0 / DIM)
            msq = rpool.tile([1, T], F32, tag="msq", name="msq")
            nc.scalar.activation(msq[:], ps_sx[:], AFT.Square, scale=1.0 / DIM)
            var = rpool.tile([1, T], F32, tag="var", name="var")
            nc.vector.scalar_tensor_tensor(var[:], ps_sxx[:], 1.0 / DIM,
                                           msq[:], ALU.mult, ALU.subtract)
            rstd = rpool.tile([1, T], BF16, tag="rstd", name="rstd")
            with nc.allow_low_precision(reason="bf16 LN rstd"):
                nc.scalar.activation(rstd[:], var[:], AFT.Abs_reciprocal_sqrt,
                                     bias=eps1[:])
            psR = psum()
            nc.tensor.matmul(psR[:], onesr1[:], rstd[:], start=True, stop=True)
            psM = psum()
            nc.tensor.matmul(psM[:], onesr1[:], m_row[:], start=True, stop=True)
            Rb = bpool.tile([128, T], BF16, tag="Rb", name="Rb")
            nc.vector.tensor_copy(Rb[:], psR[:])
            Mb = bpool.tile([128, T], BF16, tag="Mb", name="Mb")
            nc.vector.tensor_copy(Mb[:], psM[:])
            for c in range(C):
                tmp = tpool.tile([128, T], BF16, tag="tmp", name="tmp")
                nc.vector.tensor_sub(tmp[:], xb[:, ts(c, T)], Mb[:])
                with nc.allow_low_precision(reason="narrow LN output"):
                    nc.vector.tensor_mul(h[:, c * Tp:c * Tp + T], tmp[:], Rb[:])

        # ---- patch embed: x = convW^T @ patches + pe
        for m in range(C):
            ps = psum()
            for c in range(C):
                nc.tensor.matmul(
                    ps[:], cw_sb[:, c * DIM + m * 128:c * DIM + m * 128 + 128],
                    px_sb[:, ts(c, T)], start=(c == 0), stop=False)
            nc.tensor.matmul(ps[:], pe_sb[:, ts(m, 128)], ri_sb[:],
                             start=False, stop=True)
            nc.vector.tensor_copy(x[:, ts(m, T)], ps[:])
        preload(AFT.Abs_reciprocal_sqrt, x[0:1, 0:1])

        for L in range(DEPTH):
            b1c = lp_pool.tile([128, FC], F32, tag="b1c", name="b1c")
            nc.sync.dma_start(b1c[:], b1_d[L])
            bqk = lp_pool.tile([128, 2 * C], F32, tag="bqk", name="bqk")
            nc.sync.dma_start(bqk[:], bqk_d[L])
            if has_bv:
                bvr = lp_pool.tile([1, DIM], BF16, tag="bvr", name="bvr")
                nc.sync.dma_start(bvr[:], bv_d[L])
            if has_b2:
                b2p = lp_pool.tile([1, DIM], BF16, tag="b2p", name="b2p")
                nc.sync.dma_start(b2p[:], b2_d[L])

            # ---- LN1 (scale/bias folded into qkv weights host-side)
            h = hpool.tile([128, C * Tp], att_dt, tag="h", name="h")
            layernorm(h, att_dt)

            # ---- q, k (feature-major [768, 392], kept at ws scale)
            qw = wq_pool.tile([128, 6 * DIM] if FP8_ATT else [128, C * DIM],
                              att_dt, tag="wqkv", name="wqkv")
            nc.sync.dma_start(qw[:], qw_d[L])
            kw = wq_pool.tile([128, 6 * DIM] if FP8_ATT else [128, C * DIM],
                              att_dt, tag="wqkv", name="wqkv")
            nc.sync.dma_start(kw[:], kw_d[L])

            def qk_mm(dst, w, bcol):
                for m in range(C):
                    ps = psum()
                    if FP8_ATT:
                        for d in range(3):
                            wp = w[:, d * 2 * DIM:(d + 1) * 2 * DIM].rearrange(
                                "p (j o) -> p j o", j=2)[:, :, ts(m, 128)]
                            nc.tensor.matmul(ps[:], wp, pair_ap(h, d, 0, T),
                                             start=(d == 0), stop=(d == 2),
                                             perf_mode=DR)
                    else:
                        for c in range(C):
                            nc.tensor.matmul(
                                ps[:],
                                w[:, c * DIM + m * 128:c * DIM + m * 128 + 128],
                                h[:, c * Tp:c * Tp + T],
                                start=(c == 0), stop=(c == C - 1))
                    with nc.allow_low_precision(reason="bf16 q/k"):
                        nc.scalar.activation(dst[:, ts(m, T)], ps[:],
                                             AFT.Identity, bias=bcol[:, m:m + 1])
            qk_mm(qs, qw, bqk[:, 0:C])
            preload(AFT.Exp, qs[0:1, 0:1])
            qk_mm(ks, kw, bqk[:, C:2 * C])

            vw = wq_pool.tile([128, 6 * DIM] if FP8_ATT else [128, C * DIM],
                              att_dt, tag="wqkv", name="wqkv")
            nc.sync.dma_start(vw[:], vw_d[L])

            # ---- vT per image: psv [49, 384] per half
            vT = [None] * BL
            for b in range(BL):
                vT[b] = vpool.tile([KEEP, DIM], BF16, tag="vT", name="vT")
                for half in range(2):
                    psv = psum((KEEP, DIM // 2))
                    if FP8_ATT:
                        for d in range(3):
                            nc.tensor.matmul(
                                psv[:], pair_ap(h, d, b * KEEP, KEEP),
                                vw[:, d * 2 * DIM:(d + 1) * 2 * DIM].rearrange(
                                    "p (j o) -> p j o", j=2)[:, :, ts(half, 384)],
                                start=(d == 0), stop=False if has_bv else (d == 2),
                                perf_mode=DR)
                    else:
                        for c in range(C):
                            nc.tensor.matmul(
                                psv[:],
                                h[:, c * Tp + b * KEEP:c * Tp + b * KEEP + KEEP],
                                vw[:, c * DIM + half * 384:c * DIM + half * 384 + 384],
                                start=(c == 0), stop=False if has_bv else (c == C - 1))
                    if has_bv:
                        nc.tensor.matmul(psv[:], ones98r[:, 0:KEEP],
                                         bvr[:, ts(half, 384)],
                                         start=False, stop=True)
                    with nc.allow_low_precision(reason="bf16 vT"):
                        nc.scalar.activation(vT[b][:, ts(half, 384)], psv[:],
                                             AFT.Identity,
                                             scale=AS / att_ws)

            # ---- attention: scores + exp + Z cols + iz transpose
            Etl = [None] * BL
            izr = [None] * BL
            W98 = 2 * KEEP

            def stage_scores(b):
                pss = [psum((KEEP, 6 * KEEP)), psum((KEEP, 6 * KEEP))]
                for hh in range(NH):
                    j, par = hh // 2, hh % 2
                    nc.tensor.matmul(
                        pss[par][:, ts(j, KEEP)],
                        ks[64 * par:64 * par + 64,
                           j * T + b * KEEP:j * T + b * KEEP + KEEP],
                        qs[64 * par:64 * par + 64,
                           j * T + b * KEEP:j * T + b * KEEP + KEEP],
                        start=True, stop=True)
                E0 = epool.tile([KEEP, 6 * KEEP], BF16, tag="E", name="E")
                E1 = epool.tile([KEEP, 6 * KEEP], BF16, tag="E", name="E")
                nc.scalar.activation(E0[:], pss[0][:], AFT.Exp, scale=exp_scale)
                nc.scalar.activation(E1[:], pss[1][:], AFT.Exp, scale=exp_scale)
                Etl[b] = (E0, E1)

            def stage_z(b):
                zc = psum((128, 8))
                for par in range(2):
                    for c3 in range(3):
                        nc.tensor.matmul(
                            zc[0:W98, par * 3 + c3:par * 3 + c3 + 1],
                            Etl[b][par][:, c3 * W98:(c3 + 1) * W98],
                            ones49[:], start=True, stop=True)
                izc = zpool.tile([128, 8], BF16, tag="izc", name="izc")
                with nc.allow_low_precision(reason="bf16 softmax recip"):
                    nc.vector.reciprocal(izc[0:W98, 0:6], zc[0:W98, 0:6])
                pst = psum((1, 6 * W98), BF16)
                for i in range(6):
                    nc.tensor.transpose(pst[0:1, i * W98:(i + 1) * W98],
                                        izc[0:W98, i:i + 1], eye[0:W98, 0:W98])
                izr[b] = zpool.tile([1, 6 * W98], BF16, tag="izT", name="izT")
                nc.vector.tensor_copy(izr[b][:], pst[:])

            for b in range(BL):
                stage_scores(b)
                if b >= 1:
                    stage_z(b - 1)
            stage_z(BL - 1)

            # ---- AV with fused 1/Z scaling into aO
            for b in range(BL):
                izb = bpool.tile([128, 6 * W98], BF16, tag="izb", name="izb")
                nc.gpsimd.partition_broadcast(izb[:], izr[b][:])
                pso = psum((128, 6 * KEEP))
                for j in range(C):
                    for par in range(2):
                        hh = 2 * j + par
                        nc.tensor.matmul(
                            pso[64 * par:64 * par + 64, ts(j, KEEP)],
                            vT[b][:, ts(hh, 64)],
                            Etl[b][par][:, ts(j, KEEP)],
                            start=True, stop=True)
                a3 = aO[:].rearrange("p (c t) -> p c t", c=C)[:, :, b * KEEP:
                                                             (b + 1) * KEEP]
                p3 = pso[:].rearrange("p (c t) -> p c t", c=C)
                with nc.allow_low_precision(reason="narrow attention out"):
                    nc.vector.tensor_tensor(
                        a3[0:64], p3[0:64],
                        izb[0:64, 0:294].rearrange("p (c t) -> p c t", c=C),
                        ALU.mult)
                    nc.vector.tensor_tensor(
                        a3[64:128], p3[64:128],
                        izb[64:128, 294:588].rearrange("p (c t) -> p c t", c=C),
                        ALU.mult)

            # ---- proj + residual
            pw = wq_pool.tile([128, 6 * DIM] if FP8_ATT else [128, C * DIM],
                              att_dt, tag="wp", name="wp")
            nc.sync.dma_start(pw[:], pw_d[L])
            preload(AFT.Abs_reciprocal_sqrt, aO[0:1, 0:1])
            for m in range(C):
                ps = psum()
                if FP8_ATT:
                    for d in range(3):
                        nc.tensor.matmul(
                            ps[:],
                            pw[:, d * 2 * DIM:(d + 1) * 2 * DIM].rearrange(
                                "p (j o) -> p j o", j=2)[:, :, ts(m, 128)],
                            pair_ap(aO, d, 0, T),
                            start=(d == 0), stop=(d == 2), perf_mode=DR)
                else:
                    for c in range(C):
                        nc.tensor.matmul(
                            ps[:],
                            pw[:, c * DIM + m * 128:c * DIM + m * 128 + 128],
                            aO[:, c * Tp:c * Tp + T],
                            start=(c == 0), stop=(c == C - 1))
                nc.vector.scalar_tensor_tensor(
                    x[:, ts(m, T)], ps[:], 1.0 / (att_ws * AS),
                    x[:, ts(m, T)], ALU.mult, ALU.add)

            # ---- LN2 (scale/bias folded into w1/b1)
            h2 = hpool.tile([128, C * Tp], ffn_dt, tag="h", name="h")
            layernorm(h2, ffn_dt)

            # ---- FFN phase 1: g = gelu(h2 @ w1 + b1)
            if FP8_FFN:
                w1t = w1_pool.tile([128, 6 * FF], FP8, tag="w1", name="w1")
                nc.sync.dma_start(w1t[:], w1_d[L])
            for c in range(FC):
                if not FP8_FFN and c % 6 == 0:
                    w1t = w1_pool.tile([128, C * DIM], BF16, tag="w1", name="w1")
                    nc.sync.dma_start(w1t[:], w1_d[L, c // 6])
                psf = psum()
                if FP8_FFN:
                    for d in range(3):
                        nc.tensor.matmul(
                            psf[:],
                            w1t[:, d * 2 * FF:(d + 1) * 2 * FF].rearrange(
                                "p (j o) -> p j o", j=2)[:, :, ts(c, 128)],
                            pair_ap(h2, d, 0, T),
                            start=(d == 0), stop=(d == 2), perf_mode=DR)
                else:
                    for ci in range(C):
                        nc.tensor.matmul(
                            psf[:],
                            w1t[:, ci * DIM + (c % 6) * 128:
                                ci * DIM + (c % 6) * 128 + 128],
                            h2[:, ci * Tp:ci * Tp + T],
                            start=(ci == 0), stop=(ci == C - 1))
                with nc.allow_low_precision(reason="narrow gelu out"):
                    nc.scalar.activation(g[:, c * Tp:c * Tp + T], psf[:],
                                         AFT.Gelu_apprx_tanh,
                                         bias=b1c[:, c:c + 1], scale=1.0 / ffn_ws)

            # ---- FFN phase 2: x += g @ w2 + b2
            w2t = w2_pool.tile([128, W2COLS], ffn_dt, tag="w2", name="w2")
            nc.sync.dma_start(w2t[:], w2_d[L])
            for m in range(C):
                ps = psum()
                first = True
                if has_b2:
                    nc.tensor.matmul(ps[:], b2p[:, ts(m, 128)], onesrT[:],
                                     start=True, stop=False)
                    first = False
                if FP8_FFN:
                    for d in range(12):
                        nc.tensor.matmul(
                            ps[:],
                            w2t[:, d * 2 * DIM:(d + 1) * 2 * DIM].rearrange(
                                "p (j o) -> p j o", j=2)[:, :, ts(m, 128)],
                            pair_ap(g, d, 0, T),
                            start=first and (d == 0), stop=(d == 11),
                            perf_mode=DR)
                else:
                    for c in range(FC):
                        nc.tensor.matmul(
                            ps[:],
                            w2t[:, c * DIM + m * 128:c * DIM + m * 128 + 128],
                            g[:, c * Tp:c * Tp + T],
                            start=first and (c == 0), stop=(c == FC - 1))
                nc.vector.scalar_tensor_tensor(
                    x[:, ts(m, T)], ps[:], 1.0 / ffn_ws,
                    x[:, ts(m, T)], ALU.mult, ALU.add)
                if m == 0 and L + 1 < DEPTH:
                    preload(AFT.Abs_reciprocal_sqrt, x[0:1, 0:1])

        for c in range(C):
            nc.sync.dma_start(out_d[c * 128:(c + 1) * 128, :], x[:, ts(c, T)])
        es.close()

    nc.compile()
    return nc


def _prep_shared(inputs):
    """Host-side packing of weights (shared across cores)."""
    sh = {}
    sh["convw"] = _chunk_pack(
        np.asarray(inputs["conv_w"], np.float32).reshape(P * P * 3, DIM),
        DIM).astype(bf16)
    sh["eye"] = np.eye(128, dtype=np.float32).astype(bf16)

    qkv = np.asarray(inputs["qkv_w"], np.float32)
    ln1s = np.asarray(inputs["ln1_s"], np.float32)
    ln1b = np.asarray(inputs["ln1_b"], np.float32)
    ln2s = np.asarray(inputs["ln2_s"], np.float32)
    ln2b = np.asarray(inputs["ln2_b"], np.float32)
    w1 = np.asarray(inputs["w1"], np.float32)
    b1 = np.asarray(inputs["b1"], np.float32)
    w2 = np.asarray(inputs["w2"], np.float32)
    b2 = np.asarray(inputs["b2"], np.float32)
    pwt = np.asarray(inputs["proj_w"], np.float32)

    att_ws = WS if FP8_ATT else 1.0
    ffn_ws = WS if FP8_FFN else 1.0

    qw, kw, vw, pw, w1q, w2q, b1c, bqk, bvr = [], [], [], [], [], [], [], [], []
    for L in range(DEPTH):
        Wq = qkv[L, :, :DIM] * ln1s[L][:, None]
        Wk = qkv[L, :, DIM:2 * DIM] * ln1s[L][:, None]
        Wv = qkv[L, :, 2 * DIM:] * ln1s[L][:, None]
        if FP8_ATT:
            qw.append(_dr_pack(Wq, WS))
            kw.append(_dr_pack(Wk, WS))
            vw.append(_dr_pack(Wv, WS))
            pw.append(_dr_pack(pwt[L], WS))
        else:
            qw.append(_chunk_pack(Wq, DIM).astype(bf16))
            kw.append(_chunk_pack(Wk, DIM).astype(bf16))
            vw.append(_chunk_pack(Wv, DIM).astype(bf16))
            pw.append(_chunk_pack(pwt[L], DIM).astype(bf16))
        bq = ln1b[L] @ qkv[L, :, :DIM]
        bk = ln1b[L] @ qkv[L, :, DIM:2 * DIM]
        bqk.append(np.concatenate(
            [_col_pack(bq * att_ws), _col_pack(bk * att_ws)], axis=1))
        bvr.append((ln1b[L] @ qkv[L, :, 2 * DIM:]) * att_ws)

        W1 = w1[L] * ln2s[L][:, None]
        if FP8_FFN:
            w1q.append(_dr_pack(W1, WS))
            w2q.append(_dr_pack(w2[L], WS))
        else:
            w1q.append(np.stack(
                [_chunk_pack(W1[:, 768 * qt:768 * (qt + 1)], DIM)
                 for qt in range(4)]).astype(bf16))
            w2q.append(np.concatenate(
                [_chunk_pack(w2[L][768 * qt:768 * (qt + 1), :], DIM)
                 for qt in range(4)], axis=1).astype(bf16))
        b1c.append(np.ascontiguousarray(
            (b1[L] + ln2b[L] @ w1[L]).reshape(FC, 128).T))

    sh["qw"] = np.stack(qw)
    sh["kw"] = np.stack(kw)
    sh["vw"] = np.stack(vw)
    sh["pw"] = np.stack(pw)
    sh["w1q"] = np.stack(w1q)
    sh["w2q"] = np.stack(w2q)
    sh["b1c"] = np.stack(b1c).astype(np.float32)
    sh["bqk"] = np.stack(bqk).astype(np.float32)

    bvr = np.stack(bvr)
    has_bv = bool(np.any(bvr != 0.0))
    if has_bv:
        sh["bvr"] = bvr.reshape(DEPTH, 1, DIM).astype(bf16)
    has_b2 = bool(np.any(b2 != 0.0))
    if has_b2:
        sh["b2p"] = (b2 * ffn_ws).reshape(DEPTH, 1, DIM).astype(bf16)
    return sh, has_bv, has_b2


def kernel(**inputs) -> np.ndarray:
    sh, has_bv, has_b2 = _prep_shared(inputs)
    key = ("nc", FP8_FFN, FP8_ATT, has_bv, has_b2)
    if key not in _cached:
        _cached[key] = _build(has_bv, has_b2)
    nc = _cached[key]

    mask = np.minimum(np.asarray(inputs["mask_idx"]).astype(np.int64), 196)
    mask = np.maximum(mask, 1)
    pidx = mask - 1
    imgs = np.asarray(inputs["inputs"], np.float32)
    pat = imgs.reshape(B, 14, P, 14, P, 3).transpose(0, 1, 3, 2, 4, 5)
    pat = pat.reshape(B, 196, P * P * 3)[:, pidx]          # [B, 49, 768]
    pe = np.asarray(inputs["pe"], np.float32)[0, mask]     # [49, 768]

    sh["peL"] = pe.astype(bf16)
    sh["repI"] = np.tile(np.eye(KEEP, dtype=np.float32), (1, BL)).astype(bf16)

    in_maps = []
    for r in range(NCORES):
        pxT = pat[r * BL:(r + 1) * BL].reshape(T, P * P * 3).T   # [768, 392]
        m = dict(sh)
        m["pxT"] = _chunk_pack(np.ascontiguousarray(pxT), T).astype(bf16)
        in_maps.append(m)

    res = run_bass_kernel_spmd(nc, in_maps, core_ids=list(range(NCORES)),
                               trace=_cached.get("trace", False),
                               tmpdir=_cached.get("tmpdir"))
    _cached["last_result"] = res

    out = np.empty((B, KEEP, DIM), np.float32)
    for r in range(NCORES):
        o = res.results[r]["out"]                          # [768, 392]
        out[r * BL:(r + 1) * BL] = o.T.reshape(BL, KEEP, DIM)
    return out
